# revision 43
# baseline (speedup 1.0000x reference)
"""Bass/Trainium2 kernel for nn_KernelAMController (retrieval_knn).

Math: out(b,:) = -sum_g w(b,g)*mask[tb,g]*adj[tb(b),g,:] / (sum_g w*mask + eps)
with w(b,g) = exp(-2*||x_b - p_g||^2).

The Gaussian kernel (bandwidth 0.5) is spatially local: grid points beyond
~1.8 units contribute < 3e-4 relative error. Samples are k-d sorted (host)
into 64 leaves of 512 spatially-coherent queries; each leaf only visits the
grid chunks covering its bounding box + margin (~2.5 of 20 chunks). Per
512-sample group on device:
  mm1: exponent(g,b) = Pa^T @ Xa over the leaf's local grid chunks (fp16
       hi/lo split of the quadratic expansion, K=15, built on host).
  exp: ScalarE activation PSUM->SBUF fp16.
  mm2: py[64, 512] += Ct_chunk^T @ W_chunk accumulated in PSUM, columns
       m = d*20+k holding [mask*adj_x | mask*adj_y | mask] per time bin k.
Device streams py out as fp16; the host does the per-sample time-bin
selection and the final -num/(den+eps) divide (O(B) epilogue).

Chunk counts per group slot are data-dependent (computed from the k-d
leaves at call time) and baked into the compiled program; all 8 cores run
the identical program on their own leaf data (SPMD).
"""
import numpy as np
import ml_dtypes

import concourse.bass as bass
import concourse.tile as tile
from concourse import mybir, bacc
from concourse.bass_utils import run_bass_kernel_spmd

F32 = mybir.dt.float32
FP16 = mybir.dt.float16
FP16_NP = ml_dtypes.float16 if hasattr(ml_dtypes, "float16") else np.float16

B = 32768
G = 2500
GSIZE = 50
NBINS = 20
NCORES = 8
BC = B // NCORES   # 4096 samples per core
NGRP = 8           # groups (leaves) per core
BG = BC // NGRP    # 512 samples per leaf
NLEAF = NCORES * NGRP
EPS = 1e-10
MARGIN = 1.4       # neighborhood radius: truncation rel err ~4e-3
PAD_EXP = -60000.0  # fp16-representable; exp() -> 0
CB_CLAMP = 20.0    # max per-sample exponent normalization

_CACHE = {}


def _build_nc(caps):
    T = int(sum(caps))
    offs = np.concatenate([[0], np.cumsum(caps)]).astype(int)
    grp_of = np.repeat(np.arange(NGRP), caps)

    nc = bacc.Bacc("TRN2", target_bir_lowering=False)
    xa_d = nc.dram_tensor("xa", [15, BC], FP16, kind="ExternalInput")
    pa_d = nc.dram_tensor("pa", [15, T * 128], FP16, kind="ExternalInput")
    ct_d = nc.dram_tensor("ct", [128, T * 64], FP16, kind="ExternalInput")
    o_d = nc.dram_tensor("o", [NGRP, 64, BG], FP16, kind="ExternalOutput")

    NPAIR = (T + 1) // 2
    with tile.TileContext(nc) as tc:
        with (
            tc.tile_pool(name="consts", bufs=1) as consts,
            tc.tile_pool(name="wt", bufs=7) as wtp,
            tc.tile_pool(name="pw", bufs=3, space="PSUM") as pwp,
            tc.tile_pool(name="py", bufs=2, space="PSUM") as pyp,
        ):
            # Input DMAs on three engines in parallel, head pieces first so
            # the first matmuls start while the bulk still streams in.
            HD = min(6, T)          # ct head: first 6 chunks
            pa_sb = consts.tile([15, T * 128], FP16)
            nc.sync.dma_start(out=pa_sb[:, 0:2 * 128], in_=pa_d[:, 0:2 * 128])
            nc.sync.dma_start(out=pa_sb[:, 2 * 128:], in_=pa_d[:, 2 * 128:])
            xa_sb = consts.tile([15, BC], FP16)
            nc.gpsimd.dma_start(out=xa_sb[:, 0:2 * BG], in_=xa_d[:, 0:2 * BG])
            ct_sb = consts.tile([128, T * 64], FP16)
            nc.gpsimd.dma_start(out=ct_sb[:, 0:HD * 64], in_=ct_d[:, 0:HD * 64])
            nc.gpsimd.dma_start(out=xa_sb[:, 2 * BG:], in_=xa_d[:, 2 * BG:])
            if T > HD:
                nc.gpsimd.dma_start(out=ct_sb[:, HD * 64:], in_=ct_d[:, HD * 64:])
            out_sb = consts.tile([64, NGRP, BG], FP16)

            # PE clock warmup: dummy matmuls on zeroed scratch fill the
            # input-DMA wait and build the ~3us continuous-busy streak the
            # tensor engine needs to reach its full p-state.
            scratch = consts.tile([15, BG], FP16)
            nc.vector.memset(scratch[:], 0)
            sink = consts.tile([1, 8], F32)
            pyd = pyp.tile([64, BG], F32, name="py")
            for _ in range(4):
                nc.tensor.matmul(pyd[:], lhsT=scratch[:, 0:64],
                                 rhs=scratch[:], start=True, stop=True)
            nc.vector.tensor_copy(sink[:], pyd[0:1, 0:8])

            py_tiles = [None] * NGRP
            pend = []

            def emit_mm2(c, wt):
                g = int(grp_of[c])
                if c == offs[g]:
                    py_tiles[g] = pyp.tile([64, BG], F32, name="py")
                last = c == offs[g + 1] - 1
                nc.tensor.matmul(
                    py_tiles[g][:], lhsT=ct_sb[:, c * 64:(c + 1) * 64],
                    rhs=wt[:], start=(c == offs[g]), stop=last)
                if last:
                    nc.vector.tensor_copy(out_sb[:, g, :], py_tiles[g][:])
                    nc.sync.dma_start(out=o_d[g], in_=out_sb[:, g, :])

            for q in range(NPAIR):
                w = min(2, T - 2 * q)
                pw = pwp.tile([128, 2, BG], F32, name="pw")
                for j in range(w):
                    c = 2 * q + j
                    g = int(grp_of[c])
                    nc.tensor.matmul(
                        pw[:, j, :], lhsT=pa_sb[:, c * 128:(c + 1) * 128],
                        rhs=xa_sb[:, g * BG:(g + 1) * BG],
                        start=True, stop=True)
                wt = wtp.tile([128, 2, BG], FP16, name="wt")
                nc.scalar.activation(wt[:, 0:w, :], pw[:, 0:w, :],
                                     mybir.ActivationFunctionType.Exp)
                pend.append((q, w, wt))
                if len(pend) > 3:
                    qp, wp_, wtp_ = pend.pop(0)
                    for j in range(wp_):
                        emit_mm2(2 * qp + j, wtp_[:, j, :])
            for qp, wp_, wtp_ in pend:
                for j in range(wp_):
                    emit_mm2(2 * qp + j, wtp_[:, j, :])
    nc.compile()
    return nc


def _split_leaves(x):
    """Longest-axis k-d median split into 64 leaves of 512 sample indices."""
    leaves = []

    def rec(idx):
        if len(idx) == BG:
            leaves.append(idx)
            return
        xc = np.clip(x[idx], -8.3, 8.3)
        ax = int(np.argmax(xc.max(0) - xc.min(0)))
        order = np.argsort(x[idx, ax], kind="stable")
        h = len(idx) // 2
        rec(idx[order[:h]])
        rec(idx[order[h:]])

    rec(np.arange(x.shape[0]))
    return leaves


def _hi_lo(v):
    hi = v.astype(FP16_NP)
    lo = (v - hi.astype(np.float32)).astype(FP16_NP)
    return hi, lo


def kernel(t, x, grid_points, grid_adjoints, t_edges, grid_counts,
           trace=False, tmpdir=None):
    t = np.asarray(t, np.float32).reshape(B)
    x = np.asarray(x, np.float32)
    gp = np.asarray(grid_points, np.float32)
    adj = np.asarray(grid_adjoints, np.float32)
    te = np.asarray(t_edges, np.float32)
    cnt = np.asarray(grid_counts)

    tb = np.clip(np.searchsorted(te[1:-1], t, side="left"), 0, NBINS - 1)
    lin = gp[:GSIZE, 1]  # linspace(-8, 8, 50): y varies fastest (ij indexing)
    h = float(lin[1] - lin[0])

    # Per-sample exponent normalization c_b = min(2*d^2(nearest grid pt), 20):
    # keeps each sample's max weight near 1 so fp16 W never underflows for
    # spatial outliers. num/den both scale by exp(c_b); the host divide uses
    # eps*exp(c_b) so the result is exactly the reference ratio.
    gnear = np.clip(np.round((x - lin[0]) / h), 0, GSIZE - 1) * h + lin[0]
    cb = np.minimum(2.0 * ((x - gnear) ** 2).sum(1), CB_CLAMP)
    c16 = cb.astype(FP16_NP).astype(np.float32)

    leaves = _split_leaves(x)

    # per-leaf grid neighborhood (index box) and chunk count
    boxes, nchunks = [], []
    for idx in leaves:
        lo = x[idx].min(0) - MARGIN
        hi = x[idx].max(0) + MARGIN
        i0 = int(np.clip(np.searchsorted(lin, lo[0], "left"), 0, GSIZE - 1))
        i1 = int(np.clip(np.searchsorted(lin, hi[0], "right"), i0 + 1, GSIZE))
        j0 = int(np.clip(np.searchsorted(lin, lo[1], "left"), 0, GSIZE - 1))
        j1 = int(np.clip(np.searchsorted(lin, hi[1], "right"), j0 + 1, GSIZE))
        boxes.append((i0, i1, j0, j1))
        nchunks.append(-(-((i1 - i0) * (j1 - j0)) // 128))

    # slot s takes the 8 leaves ranked [8s, 8s+8) by descending chunk count;
    # its capacity is the max in the slot, so all cores share one program.
    order = np.argsort(-np.array(nchunks), kind="stable")
    caps = tuple(int(nchunks[order[8 * s]]) for s in range(NGRP))
    T = sum(caps)
    assign = [[int(order[8 * s + c]) for s in range(NGRP)] for c in range(NCORES)]

    # precompute full-grid quadratic expansion (f32) and ct rows (f32)
    p5 = np.empty((5, G), np.float32)
    p5[0] = 4.0 * gp[:, 0]
    p5[1] = 4.0 * gp[:, 1]
    p5[2] = -2.0
    p5[3] = -2.0
    p5[4] = -2.0 * (gp[:, 0] ** 2 + gp[:, 1] ** 2)
    mask = (cnt > 0).astype(np.float32)                 # (20, G)
    ct_full = np.empty((G, 64), np.float32)
    ct_full[:, 0:20] = (mask * adj[:, :, 0]).T
    ct_full[:, 20:40] = (mask * adj[:, :, 1]).T
    ct_full[:, 40:60] = mask.T
    ct_full[:, 60:64] = 0.0

    in_maps = []
    for c in range(NCORES):
        xa = np.zeros((15, BC), np.float32)
        pa = np.zeros((15, T * 128), FP16_NP)
        pa[4] = PAD_EXP
        pa[9] = 1.0
        ct = np.zeros((T * 128, 64), FP16_NP)
        off = 0
        for s in range(NGRP):
            li = assign[c][s]
            idx = leaves[li]
            i0, i1, j0, j1 = boxes[li]
            ii, jj = np.meshgrid(np.arange(i0, i1), np.arange(j0, j1),
                                 indexing="ij")
            gidx = (ii * GSIZE + jj).reshape(-1)
            n = len(gidx)

            xs = x[idx]
            sl = slice(s * BG, (s + 1) * BG)
            x_hi, x_lo = _hi_lo(xs.T)                   # (2, BG)
            sq_hi, sq_lo = _hi_lo(xs.T.astype(np.float32) ** 2)
            xa[0:2, sl] = x_hi
            xa[2:4, sl] = sq_hi
            xa[4, sl] = 1.0
            xa[5:7, sl] = x_lo
            xa[7:9, sl] = sq_lo
            xa[9, sl] = c16[idx]
            xa[10:12, sl] = x_hi
            xa[12:14, sl] = sq_hi
            xa[14, sl] = 1.0

            p_hi, p_lo = _hi_lo(p5[:, gidx])            # (5, n)
            pa[0:5, 128 * off:128 * off + n] = p_hi
            pa[5:9, 128 * off:128 * off + n] = p_hi[0:4]
            pa[10:15, 128 * off:128 * off + n] = p_lo
            ct[128 * off:128 * off + n] = ct_full[gidx]
            off += caps[s]

        ct_dram = np.ascontiguousarray(
            ct.reshape(T, 128, 64).transpose(1, 0, 2).reshape(128, T * 64))
        in_maps.append({"xa": xa.astype(FP16_NP), "pa": pa, "ct": ct_dram})

    key = ("nc", caps)
    if key not in _CACHE:
        _CACHE[key] = _build_nc(caps)
    nc = _CACHE[key]
    res = run_bass_kernel_spmd(nc, in_maps, core_ids=list(range(NCORES)),
                               trace=trace, tmpdir=tmpdir)
    _CACHE["last_result"] = res

    out = np.empty((B, 2), np.float32)
    jcol = np.arange(BG)
    for c in range(NCORES):
        o = np.asarray(res.results[c]["o"]).astype(np.float32)  # (NGRP, 64, BG)
        for s in range(NGRP):
            idx = leaves[assign[c][s]]
            blk = o[s]
            k = tb[idx]
            den = blk[40 + k, jcol] + EPS * np.exp(c16[idx])
            out[idx, 0] = -blk[k, jcol] / den
            out[idx, 1] = -blk[20 + k, jcol] / den
    return out
